# revision 8
# baseline (speedup 1.0000x reference)
"""Trainium2 Bass kernel for nn_Dense_4277787427179 (per-degree block-diagonal dense).

Computation: x [B=16384, P=2, C=16, F=256] f32; for degree l in 0..3 the C-slice
[l^2, (l+1)^2) (sizes 1,3,5,7) is multiplied by W_e[l] (parity 0) / W_o[l]
(parity 1) on the feature axis; bias b added only to (parity 0, l=0).

Strategy (data-parallel over 8 NeuronCores, batch axis sharded):
- Host: per shard, regroup+transpose x to xT[f=256, r'=65536] with columns
  ordered (p, c, b) -- each (p, degree) group is then a contiguous column
  range (multiple of 2048 cols). Split fp32 into bf16 hi + bf16 lo
  (x == hi + lo to ~16-bit mantissa; same total DMA bytes as fp32).
- Device: per 128-row tile: 3 matmul passes (hi@W_hi, hi@W_lo, lo@W_hi), each
  over 2 contraction chunks, accumulated in one fp32 PSUM tile [128, 256]
  (12 PE cyc/row vs 16 for native fp32, with ~fp32 precision); copy to SBUF
  (+bias for the p=0,l=0 group); DMA out in natural row layout.
- Host: ungroup out rows (p, c, b) -> [b, p, c, g] and concat shards.

MODE: "bf16x2" (default), "f32" (exact native fp32 matmul, ~1.37x slower),
"f32r" (full-rate reduced fp32, ~1.5e-4 rel err).
"""

import numpy as np
from concurrent.futures import ThreadPoolExecutor

import ml_dtypes

import concourse.bass as bass
import concourse.mybir as mybir
import concourse.tile as tile
from concourse import bacc
from concourse.bass_utils import run_bass_kernel_spmd

N_CORES = 8
B, P, C, F = 16384, 2, 16, 256
BS = B // N_CORES           # 2048 batch per core
ROWS = BS * P * C           # 65536 rows per core
DEG_SIZES = [1, 3, 5, 7]    # c-rows per degree
RC = 4096                   # rows per xq load chunk
ST = 16                     # row-tiles (of 128) per output store

MODE = "bf16x2"
BF16 = ml_dtypes.bfloat16

_nc_cache = {}


def _build_nc_bf16x2():
    nc = bacc.Bacc("TRN2", target_bir_lowering=False, debug=False,
                   num_devices=N_CORES)
    # hi/lo stacked along the leading (split, k-chunk) axis:
    # xq[(s k p), r] with row = s*256 + kc*128 + p; wq[(s*16+m), p, g]
    xq = nc.dram_tensor("xq", [2 * F, ROWS], mybir.dt.bfloat16,
                        kind="ExternalInput").ap()
    wq = nc.dram_tensor("wq", [128, 32, 256], mybir.dt.bfloat16,
                        kind="ExternalInput").ap()
    bias = nc.dram_tensor("bias", [128, 256], mybir.dt.float32,
                          kind="ExternalInput").ap()
    # partition-major output: out[p, t, g] holds row r' = t*128 + p
    # (16 KiB contiguous DMA runs per partition line; host un-shuffles)
    out = nc.dram_tensor("out", [128, ROWS // 128, 256], mybir.dt.float32,
                         kind="ExternalOutput").ap()

    xq_v = xq.rearrange("(q p) r -> p q r", p=128)       # [128, 4, ROWS]; q = s*2+kc

    # degree of each 2048-row block (blocks ordered p, c)
    L_OF_C = [0, 1, 1, 1, 2, 2, 2, 2, 2, 3, 3, 3, 3, 3, 3, 3]

    # (x-part, w-part) per pass: hi@Wh + hi@Wl + lo@Wh
    PASSES = [(0, 0), (0, 1), (1, 0)]

    with tile.TileContext(nc) as tc:
        with (
            tc.tile_pool(name="wpool", bufs=1) as wpool,
            tc.tile_pool(name="xpool", bufs=2) as xpool,
            tc.tile_pool(name="opool", bufs=2) as opool,
            tc.tile_pool(name="pspool", bufs=8, space=bass.MemorySpace.PSUM) as pspool,
        ):
            w_sb = wpool.tile([128, 32, 256], mybir.dt.bfloat16)
            nc.scalar.dma_start(out=w_sb[:], in_=wq)
            b_sb = wpool.tile([128, 256], mybir.dt.float32)
            nc.scalar.dma_start(out=b_sb[:], in_=bias)

            chunk_sizes = [2048, 2048, 4096] + [8192] * 7
            assert sum(chunk_sizes) == ROWS
            r0 = 0
            for rc in chunk_sizes:
                xt = xpool.tile([128, 4, rc], mybir.dt.bfloat16, tag="xt")
                nc.sync.dma_start(out=xt[:], in_=xq_v[:, :, r0:r0 + rc])
                n_t = rc // 128
                for st in range(n_t // ST):
                    o_sb = opool.tile([128, ST, 256], mybir.dt.float32)
                    for j in range(ST):
                        rt = st * ST + j
                        row0 = r0 + rt * 128
                        blk = row0 // BS          # 0..31 = p*16 + c
                        p, c = blk // 16, blk % 16
                        l = L_OF_C[c]
                        add_bias = (p == 0 and c == 0)
                        ps = pspool.tile([128, 256], mybir.dt.float32)
                        n_mm = 2 * len(PASSES)
                        i_mm = 0
                        for kc in range(2):
                            m = (p * 4 + l) * 2 + kc
                            for sx, sw in PASSES:
                                nc.tensor.matmul(
                                    ps[:],
                                    lhsT=xt[:, sx * 2 + kc,
                                            rt * 128:(rt + 1) * 128],
                                    rhs=w_sb[:, sw * 16 + m, :],
                                    start=(i_mm == 0),
                                    stop=(i_mm == n_mm - 1),
                                )
                                i_mm += 1
                        if add_bias:
                            nc.vector.tensor_add(o_sb[:, j, :], ps[:], b_sb[:])
                        else:
                            nc.vector.tensor_copy(o_sb[:, j, :], ps[:])
                    t0 = r0 // 128 + st * ST
                    nc.scalar.dma_start(out=out[:, t0:t0 + ST, :], in_=o_sb[:])
                r0 += rc
    nc.compile()
    return nc


def _build_nc_f32(use_f32r):
    nc = bacc.Bacc("TRN2", target_bir_lowering=False, debug=False,
                   num_devices=N_CORES)
    io_dt = mybir.dt.float32r if use_f32r else mybir.dt.float32
    RC2, ST2 = 2048, 8

    xT = nc.dram_tensor("xT", [F, ROWS], io_dt, kind="ExternalInput").ap()
    wg = nc.dram_tensor("wg", [16, 128, 256], io_dt, kind="ExternalInput").ap()
    bias = nc.dram_tensor("bias", [128, 256], mybir.dt.float32,
                          kind="ExternalInput").ap()
    out = nc.dram_tensor("out", [ROWS, 256], mybir.dt.float32,
                         kind="ExternalOutput").ap()

    xT_v = xT.rearrange("(k p) r -> p k r", k=2)        # [128, 2, ROWS]
    out_v = out.rearrange("(t p) g -> p t g", p=128)    # [128, ROWS//128, 256]

    with tile.TileContext(nc) as tc:
        with (
            tc.tile_pool(name="wpool", bufs=1) as wpool,
            tc.tile_pool(name="xpool", bufs=2) as xpool,
            tc.tile_pool(name="opool", bufs=2) as opool,
            tc.tile_pool(name="pspool", bufs=8, space=bass.MemorySpace.PSUM) as pspool,
        ):
            w_sb = wpool.tile([128, 16, 256], io_dt)
            nc.sync.dma_start(out=w_sb[:], in_=wg.rearrange("m p g -> p m g"))
            b_sb = wpool.tile([128, 256], mybir.dt.float32)
            nc.sync.dma_start(out=b_sb[:], in_=bias)

            r0 = 0
            for p in range(P):
                for l in range(4):
                    g_rows = BS * DEG_SIZES[l]
                    add_bias = (p == 0 and l == 0)
                    for ch in range(g_rows // RC2):
                        xt = xpool.tile([128, 2, RC2], io_dt)
                        nc.sync.dma_start(out=xt[:],
                                          in_=xT_v[:, :, r0:r0 + RC2])
                        n_t = RC2 // 128
                        for st in range(n_t // ST2):
                            o_sb = opool.tile([128, ST2, 256], mybir.dt.float32)
                            for j in range(ST2):
                                rt = st * ST2 + j
                                ps = pspool.tile([128, 256], mybir.dt.float32)
                                for kc in range(2):
                                    m = (p * 4 + l) * 2 + kc
                                    nc.tensor.matmul(
                                        ps[:],
                                        lhsT=xt[:, kc, rt * 128:(rt + 1) * 128],
                                        rhs=w_sb[:, m, :],
                                        start=(kc == 0),
                                        stop=(kc == 1),
                                    )
                                if add_bias:
                                    nc.vector.tensor_add(o_sb[:, j, :], ps[:],
                                                         b_sb[:])
                                else:
                                    nc.vector.tensor_copy(o_sb[:, j, :], ps[:])
                            t0 = r0 // 128 + st * ST2
                            nc.sync.dma_start(
                                out=out_v[:, t0:t0 + ST2, :], in_=o_sb[:])
                        r0 += RC2
    nc.compile()
    return nc


def _get_nc():
    if MODE not in _nc_cache:
        if MODE == "bf16x2":
            _nc_cache[MODE] = _build_nc_bf16x2()
        else:
            _nc_cache[MODE] = _build_nc_f32(MODE == "f32r")
    return _nc_cache[MODE]


def _build_shard_xT(xs):
    """[BS, 2, 16, 256] -> xT [256, 65536] f32, columns ordered (p, c, b)."""
    y = np.ascontiguousarray(xs.transpose(1, 2, 0, 3))  # [2, 16, BS, 256]
    yv = y.reshape(P * C, BS, F)
    xT = np.empty((F, ROWS), np.float32)
    xv = xT.reshape(F, P * C, BS)
    for j in range(P * C):
        xv[:, j, :] = yv[j].T
    return xT


def _build_shard_xq(xs):
    xT = _build_shard_xT(xs)
    xq = np.empty((2 * F, ROWS), BF16)
    hi = xT.astype(BF16)
    xq[:F] = hi
    xq[F:] = (xT - hi.astype(np.float32)).astype(BF16)
    return xq


def run_sharded(x, W_e, W_o, b, trace=False):
    x = np.asarray(x, dtype=np.float32)
    W = np.stack([np.asarray(W_e, np.float32), np.asarray(W_o, np.float32)])
    wg = np.ascontiguousarray(W.reshape(2, 4, 2, 128, 256).reshape(16, 128, 256))
    bias = np.broadcast_to(np.asarray(b, np.float32).reshape(1, 256),
                           (128, 256)).copy()

    nc = _get_nc()
    shards = [x[i * BS:(i + 1) * BS] for i in range(N_CORES)]

    if MODE == "bf16x2":
        wh = wg.astype(BF16)
        wq0 = np.empty((32, 128, 256), BF16)
        wq0[:16] = wh
        wq0[16:] = (wg - wh.astype(np.float32)).astype(BF16)
        wq = np.ascontiguousarray(wq0.transpose(1, 0, 2))
        with ThreadPoolExecutor(N_CORES) as ex:
            xqs = list(ex.map(_build_shard_xq, shards))
        in_maps = [{"xq": xqs[i], "wq": wq, "bias": bias}
                   for i in range(N_CORES)]
    else:
        with ThreadPoolExecutor(N_CORES) as ex:
            xTs = list(ex.map(_build_shard_xT, shards))
        in_maps = [{"xT": xTs[i], "wg": wg, "bias": bias}
                   for i in range(N_CORES)]

    res = run_bass_kernel_spmd(nc, in_maps, core_ids=list(range(N_CORES)),
                               trace=trace)

    out = np.empty((B, P, C, F), np.float32)
    for i in range(N_CORES):
        og = res.results[i]["out"]  # [128, 512, 256]; row r' = t*128 + p
        if og.ndim == 3:
            og = og.transpose(1, 0, 2).reshape(ROWS, F)
        out[i * BS:(i + 1) * BS] = og.reshape(P, C, BS, F).transpose(2, 0, 1, 3)
    return out, res


def kernel(x, W_e, W_o, b):
    out, _ = run_sharded(x, W_e, W_o, b, trace=False)
    return out


# revision 9
# speedup vs baseline: 1.0515x; 1.0515x over previous
"""Trainium2 Bass kernel for nn_Dense_4277787427179 (per-degree block-diagonal dense).

Computation: x [B=16384, P=2, C=16, F=256] f32; for degree l in 0..3 the C-slice
[l^2, (l+1)^2) (sizes 1,3,5,7) is multiplied by W_e[l] (parity 0) / W_o[l]
(parity 1) on the feature axis; bias b added only to (parity 0, l=0).

Strategy (data-parallel over 8 NeuronCores, batch axis sharded):
- Host: per shard, regroup+transpose x to xT[f=256, r'=65536] with columns
  ordered (p, c, b) -- each (p, degree) group is then a contiguous column
  range (multiple of 2048 cols). Split fp32 into bf16 hi + bf16 lo
  (x == hi + lo to ~16-bit mantissa; same total DMA bytes as fp32).
- Device: per 128-row tile: 3 matmul passes (hi@W_hi, hi@W_lo, lo@W_hi), each
  over 2 contraction chunks, accumulated in one fp32 PSUM tile [128, 256]
  (12 PE cyc/row vs 16 for native fp32, with ~fp32 precision); copy to SBUF
  (+bias for the p=0,l=0 group); DMA out in natural row layout.
- Host: ungroup out rows (p, c, b) -> [b, p, c, g] and concat shards.

MODE: "bf16x2" (default), "f32" (exact native fp32 matmul, ~1.37x slower),
"f32r" (full-rate reduced fp32, ~1.5e-4 rel err).
"""

import numpy as np
from concurrent.futures import ThreadPoolExecutor

import ml_dtypes

import concourse.bass as bass
import concourse.mybir as mybir
import concourse.tile as tile
from concourse import bacc
from concourse.bass_utils import run_bass_kernel_spmd

N_CORES = 8
B, P, C, F = 16384, 2, 16, 256
BS = B // N_CORES           # 2048 batch per core
ROWS = BS * P * C           # 65536 rows per core
DEG_SIZES = [1, 3, 5, 7]    # c-rows per degree
RC = 4096                   # rows per xq load chunk
ST = 16                     # row-tiles (of 128) per output store

MODE = "bf16x2"
BF16 = ml_dtypes.bfloat16

_nc_cache = {}


def _build_nc_bf16x2():
    nc = bacc.Bacc("TRN2", target_bir_lowering=False, debug=False,
                   num_devices=N_CORES)
    # hi/lo stacked along the leading (split, k-chunk) axis:
    # xq[(s k p), r] with row = s*256 + kc*128 + p; wq[(s*16+m), p, g]
    xq = nc.dram_tensor("xq", [2 * F, ROWS], mybir.dt.bfloat16,
                        kind="ExternalInput").ap()
    wq = nc.dram_tensor("wq", [128, 32, 256], mybir.dt.bfloat16,
                        kind="ExternalInput").ap()
    bias = nc.dram_tensor("bias", [128, 256], mybir.dt.float32,
                          kind="ExternalInput").ap()
    # partition-major output: out[p, t, g] holds row r' = t*128 + p
    # (16 KiB contiguous DMA runs per partition line; host un-shuffles)
    out = nc.dram_tensor("out", [128, ROWS // 128, 256], mybir.dt.float32,
                         kind="ExternalOutput").ap()

    xq_v = xq.rearrange("(q p) r -> p q r", p=128)       # [128, 4, ROWS]; q = s*2+kc

    # degree of each 2048-row block (blocks ordered p, c)
    L_OF_C = [0, 1, 1, 1, 2, 2, 2, 2, 2, 3, 3, 3, 3, 3, 3, 3]

    # (x-part, w-part) per pass: hi@Wh + hi@Wl + lo@Wh
    PASSES = [(0, 0), (0, 1), (1, 0)]

    with tile.TileContext(nc) as tc:
        with (
            tc.tile_pool(name="wpool", bufs=1) as wpool,
            tc.tile_pool(name="xpool", bufs=4) as xpool,
            tc.tile_pool(name="opool", bufs=2) as opool,
            tc.tile_pool(name="pspool", bufs=8, space=bass.MemorySpace.PSUM) as pspool,
        ):
            w_sb = wpool.tile([128, 32, 256], mybir.dt.bfloat16)
            nc.scalar.dma_start(out=w_sb[:], in_=wq)
            b_sb = wpool.tile([128, 256], mybir.dt.float32)
            nc.scalar.dma_start(out=b_sb[:], in_=bias)

            chunk_sizes = [2048, 2048] + [4096] * 15
            assert sum(chunk_sizes) == ROWS
            r0 = 0
            for rc in chunk_sizes:
                xt = xpool.tile([128, 4, rc], mybir.dt.bfloat16, tag="xt")
                nc.sync.dma_start(out=xt[:], in_=xq_v[:, :, r0:r0 + rc])
                n_t = rc // 128
                for st in range(n_t // ST):
                    o_sb = opool.tile([128, ST, 256], mybir.dt.float32)
                    for j in range(ST):
                        rt = st * ST + j
                        row0 = r0 + rt * 128
                        blk = row0 // BS          # 0..31 = p*16 + c
                        p, c = blk // 16, blk % 16
                        l = L_OF_C[c]
                        add_bias = (p == 0 and c == 0)
                        ps = pspool.tile([128, 256], mybir.dt.float32)
                        n_mm = 2 * len(PASSES)
                        i_mm = 0
                        for kc in range(2):
                            m = (p * 4 + l) * 2 + kc
                            for sx, sw in PASSES:
                                nc.tensor.matmul(
                                    ps[:],
                                    lhsT=xt[:, sx * 2 + kc,
                                            rt * 128:(rt + 1) * 128],
                                    rhs=w_sb[:, sw * 16 + m, :],
                                    start=(i_mm == 0),
                                    stop=(i_mm == n_mm - 1),
                                )
                                i_mm += 1
                        if add_bias:
                            nc.vector.tensor_add(o_sb[:, j, :], ps[:], b_sb[:])
                        else:
                            nc.vector.tensor_copy(o_sb[:, j, :], ps[:])
                    t0 = r0 // 128 + st * ST
                    nc.scalar.dma_start(out=out[:, t0:t0 + ST, :], in_=o_sb[:])
                r0 += rc
    nc.compile()
    return nc


def _build_nc_f32(use_f32r):
    nc = bacc.Bacc("TRN2", target_bir_lowering=False, debug=False,
                   num_devices=N_CORES)
    io_dt = mybir.dt.float32r if use_f32r else mybir.dt.float32
    RC2, ST2 = 2048, 8

    xT = nc.dram_tensor("xT", [F, ROWS], io_dt, kind="ExternalInput").ap()
    wg = nc.dram_tensor("wg", [16, 128, 256], io_dt, kind="ExternalInput").ap()
    bias = nc.dram_tensor("bias", [128, 256], mybir.dt.float32,
                          kind="ExternalInput").ap()
    out = nc.dram_tensor("out", [ROWS, 256], mybir.dt.float32,
                         kind="ExternalOutput").ap()

    xT_v = xT.rearrange("(k p) r -> p k r", k=2)        # [128, 2, ROWS]
    out_v = out.rearrange("(t p) g -> p t g", p=128)    # [128, ROWS//128, 256]

    with tile.TileContext(nc) as tc:
        with (
            tc.tile_pool(name="wpool", bufs=1) as wpool,
            tc.tile_pool(name="xpool", bufs=4) as xpool,
            tc.tile_pool(name="opool", bufs=2) as opool,
            tc.tile_pool(name="pspool", bufs=8, space=bass.MemorySpace.PSUM) as pspool,
        ):
            w_sb = wpool.tile([128, 16, 256], io_dt)
            nc.sync.dma_start(out=w_sb[:], in_=wg.rearrange("m p g -> p m g"))
            b_sb = wpool.tile([128, 256], mybir.dt.float32)
            nc.sync.dma_start(out=b_sb[:], in_=bias)

            r0 = 0
            for p in range(P):
                for l in range(4):
                    g_rows = BS * DEG_SIZES[l]
                    add_bias = (p == 0 and l == 0)
                    for ch in range(g_rows // RC2):
                        xt = xpool.tile([128, 2, RC2], io_dt)
                        nc.sync.dma_start(out=xt[:],
                                          in_=xT_v[:, :, r0:r0 + RC2])
                        n_t = RC2 // 128
                        for st in range(n_t // ST2):
                            o_sb = opool.tile([128, ST2, 256], mybir.dt.float32)
                            for j in range(ST2):
                                rt = st * ST2 + j
                                ps = pspool.tile([128, 256], mybir.dt.float32)
                                for kc in range(2):
                                    m = (p * 4 + l) * 2 + kc
                                    nc.tensor.matmul(
                                        ps[:],
                                        lhsT=xt[:, kc, rt * 128:(rt + 1) * 128],
                                        rhs=w_sb[:, m, :],
                                        start=(kc == 0),
                                        stop=(kc == 1),
                                    )
                                if add_bias:
                                    nc.vector.tensor_add(o_sb[:, j, :], ps[:],
                                                         b_sb[:])
                                else:
                                    nc.vector.tensor_copy(o_sb[:, j, :], ps[:])
                            t0 = r0 // 128 + st * ST2
                            nc.sync.dma_start(
                                out=out_v[:, t0:t0 + ST2, :], in_=o_sb[:])
                        r0 += RC2
    nc.compile()
    return nc


def _get_nc():
    if MODE not in _nc_cache:
        if MODE == "bf16x2":
            _nc_cache[MODE] = _build_nc_bf16x2()
        else:
            _nc_cache[MODE] = _build_nc_f32(MODE == "f32r")
    return _nc_cache[MODE]


def _build_shard_xT(xs):
    """[BS, 2, 16, 256] -> xT [256, 65536] f32, columns ordered (p, c, b)."""
    y = np.ascontiguousarray(xs.transpose(1, 2, 0, 3))  # [2, 16, BS, 256]
    yv = y.reshape(P * C, BS, F)
    xT = np.empty((F, ROWS), np.float32)
    xv = xT.reshape(F, P * C, BS)
    for j in range(P * C):
        xv[:, j, :] = yv[j].T
    return xT


def _build_shard_xq(xs):
    xT = _build_shard_xT(xs)
    xq = np.empty((2 * F, ROWS), BF16)
    hi = xT.astype(BF16)
    xq[:F] = hi
    xq[F:] = (xT - hi.astype(np.float32)).astype(BF16)
    return xq


def run_sharded(x, W_e, W_o, b, trace=False):
    x = np.asarray(x, dtype=np.float32)
    W = np.stack([np.asarray(W_e, np.float32), np.asarray(W_o, np.float32)])
    wg = np.ascontiguousarray(W.reshape(2, 4, 2, 128, 256).reshape(16, 128, 256))
    bias = np.broadcast_to(np.asarray(b, np.float32).reshape(1, 256),
                           (128, 256)).copy()

    nc = _get_nc()
    shards = [x[i * BS:(i + 1) * BS] for i in range(N_CORES)]

    if MODE == "bf16x2":
        wh = wg.astype(BF16)
        wq0 = np.empty((32, 128, 256), BF16)
        wq0[:16] = wh
        wq0[16:] = (wg - wh.astype(np.float32)).astype(BF16)
        wq = np.ascontiguousarray(wq0.transpose(1, 0, 2))
        with ThreadPoolExecutor(N_CORES) as ex:
            xqs = list(ex.map(_build_shard_xq, shards))
        in_maps = [{"xq": xqs[i], "wq": wq, "bias": bias}
                   for i in range(N_CORES)]
    else:
        with ThreadPoolExecutor(N_CORES) as ex:
            xTs = list(ex.map(_build_shard_xT, shards))
        in_maps = [{"xT": xTs[i], "wg": wg, "bias": bias}
                   for i in range(N_CORES)]

    res = run_bass_kernel_spmd(nc, in_maps, core_ids=list(range(N_CORES)),
                               trace=trace)

    out = np.empty((B, P, C, F), np.float32)
    for i in range(N_CORES):
        og = res.results[i]["out"]  # [128, 512, 256]; row r' = t*128 + p
        if og.ndim == 3:
            og = og.transpose(1, 0, 2).reshape(ROWS, F)
        out[i * BS:(i + 1) * BS] = og.reshape(P, C, BS, F).transpose(2, 0, 1, 3)
    return out, res


def kernel(x, W_e, W_o, b):
    out, _ = run_sharded(x, W_e, W_o, b, trace=False)
    return out
